# revision 18
# baseline (speedup 1.0000x reference)
"""BiLSTM-CRF Trainium2 kernel — data-parallel over batch across 8 NeuronCores.

Layout (per core, BC=4 sequences):
  - Gates reordered [i, f, o, g] (8 chunks of 128 across partitions).
  - LSTM recurrence in "orientation B": g^T[gate,b] = WhhT_chunk.T @ h^T[h,b],
    gate dim on partitions => tiny-FD vector/scalar ops.
  - xg = x_aug @ Wih_aug precomputed on device (bias folded via ones row).
  - Viterbi on device (max-plus over [4, 12x12]); backpointers DMA'd out;
    final argmax + backtrace on host (pure integer index chasing).
"""

import numpy as np

V, E, H, B, T, K = 100000, 300, 512, 32, 256, 12
START, STOP = 10, 11
H2 = H // 2  # 256
NCORES = 8
BC = B // NCORES  # 4
NG = 8  # gate chunks of 128 (4 gates x 2 chunks), order [i,f,o,g]
TI = T + 2  # hs time slots: fwd init at 0, bwd init at T+1

_CACHE = {}


def _build_nc():
    import concourse.bass as bass
    import concourse.tile as tile
    from concourse import mybir
    from concourse.masks import make_identity

    f32 = mybir.dt.float32
    nc = bass.Bass()

    xT = nc.declare_dram_parameter("xT", [E + 1, BC * T], f32, isOutput=False)        # x_aug^T, col=b*T+t, row 300=ones
    wih = nc.declare_dram_parameter("wih", [E + 1, 2 * 4 * H2], f32, isOutput=False)  # col = d*1024+gc*128+..
    whhT = nc.declare_dram_parameter("whhT", [H2, 2 * 4 * H2], f32, isOutput=False)
    stT = nc.declare_dram_parameter("stT", [128, 32], f32, isOutput=False)            # h0 pack [.,0:16] (d,c,b), c0 pack [.,16:32]
    woutT = nc.declare_dram_parameter("woutT", [H + 1, K], f32, isOutput=False)       # row 512 = bout
    vit = nc.declare_dram_parameter("vit", [BC, 312], f32, isOutput=False)            # trans144 | revk144 | stop12 | alpha0 12
    out_bp = nc.declare_dram_parameter("out_bp", [BC, T * K], f32, isOutput=True)
    out_pre = nc.declare_dram_parameter("out_pre", [BC, K], f32, isOutput=True)

    GW = 4 * H2  # 1024 gates per dir

    from contextlib import ExitStack
    with tile.TileContext(nc) as tc, ExitStack() as es:
        const = es.enter_context(tc.tile_pool(name="const", bufs=1))
        sbuf = es.enter_context(tc.tile_pool(name="sbuf", bufs=3))
        ps1 = es.enter_context(tc.tile_pool(name="ps1", bufs=2, space="PSUM"))
        ps2 = es.enter_context(tc.tile_pool(name="ps2", bufs=2, space="PSUM"))
        ps3 = es.enter_context(tc.tile_pool(name="ps3", bufs=1, space="PSUM"))

        # ---- persistent SBUF tensors ----
        ident = const.tile([128, 128], f32)
        make_identity(nc, ident[:])
        ones_sb = const.tile([1, 512], f32)
        nc.gpsimd.memset(ones_sb[:], 1.0)

        xT_sb = [const.tile([128, BC * T], f32, name=f"xT{i}") for i in range(2)] + [
            const.tile([45, BC * T], f32, name="xT2")]
        wih_sb = [const.tile([128, 2 * GW], f32, name=f"wih{i}") for i in range(2)] + [
            const.tile([45, 2 * GW], f32, name="wih2")]
        whh_sb = [const.tile([128, 2 * GW], f32, name=f"whh{i}") for i in range(2)]
        wout_sb = const.tile([128, 4 * K], f32)   # 4 chunks side by side [128, (chunk, K)]
        wout_b = const.tile([1, K], f32)
        vit_sb = const.tile([BC, 312], f32)
        xg_sb = const.tile([128, 2 * NG * BC * T], f32)   # (d, gc, b, t)
        hs_sb = const.tile([128, 2 * 2 * BC * TI], f32)   # (d, c, b, i) i=t+1 state-after-token
        cT_sb = const.tile([128, 16], f32)                # (d, c, b)
        feats_sb = const.tile([BC, T * K], f32)           # [4, (t, j)]
        bp_sb = const.tile([BC, T * K], f32)
        pre_sb = const.tile([BC, K], f32)

        rows = [(0, 128), (128, 128), (256, 45)]
        for i, (r0, nr) in enumerate(rows):
            nc.sync.dma_start(out=xT_sb[i][:nr, :], in_=xT[r0:r0 + nr, :])
            nc.sync.dma_start(out=wih_sb[i][:nr, :], in_=wih[r0:r0 + nr, :])
        for c in range(2):
            nc.sync.dma_start(out=whh_sb[c][:], in_=whhT[c * 128:(c + 1) * 128, :])
            # wout chunks: rows (c)*128 fwd, 256+(c)*128 bwd -> cols layout (d*2+c)
            for d in range(2):
                nc.sync.dma_start(out=wout_sb[:, (d * 2 + c) * K:(d * 2 + c + 1) * K],
                                  in_=woutT[(d * 2 + c) * 128:(d * 2 + c + 1) * 128, :])
        nc.sync.dma_start(out=wout_b[:], in_=woutT[H:H + 1, :])
        nc.sync.dma_start(out=vit_sb[:], in_=vit[:])

        hsv = hs_sb[:].rearrange("p (d c b i) -> p d c b i", d=2, c=2, b=BC)
        xgv = xg_sb[:].rearrange("p (d g b t) -> p d g b t", d=2, g=NG, b=BC)

        # init states: h0 fwd -> i=0, h0 bwd -> i=T+1, c0 -> cT_sb
        stv = stT[:].rearrange("p (x d c b) -> p x d c b", x=2, d=2, c=2)
        nc.sync.dma_start(out=hsv[:, 0, :, :, 0], in_=stv[:, 0, 0, :, :])
        nc.sync.dma_start(out=hsv[:, 1, :, :, T + 1], in_=stv[:, 0, 1, :, :])
        nc.sync.dma_start(out=cT_sb[:], in_=stT[:, 16:32])

        # PE probes: touch each DMA'd/engine-written tensor once so the PE
        # vector clock observes every producer sem; real Matmults then need
        # <=1 wait (LDWEIGHTS ISA struct holds only one sync wait).
        probe_ps = ps3.tile([1, 1], f32, name="probe_ps")
        probe_srcs = [xT_sb[0], xT_sb[1], xT_sb[2], wih_sb[0], wih_sb[1], wih_sb[2],
                      whh_sb[0], whh_sb[1], wout_sb[:, 0:1], wout_sb[:, 12:13],
                      wout_sb[:, 24:25], wout_sb[:, 36:37], wout_b, ones_sb, ident]
        for ap in probe_srcs:
            a = ap[0:1, 0:1]
            nc.tensor.matmul(out=probe_ps[:], lhsT=a, rhs=a,
                             start=True, stop=True, skip_group_check=True)
        nc.tensor.matmul(out=probe_ps[:], lhsT=hsv[0:1, 0, 0, 0:1, 0],
                         rhs=hsv[0:1, 0, 0, 0:1, 0], start=True, stop=True,
                         skip_group_check=True)
        nc.tensor.matmul(out=probe_ps[:], lhsT=hsv[0:1, 1, 0, 0:1, T + 1],
                         rhs=hsv[0:1, 1, 0, 0:1, T + 1], start=True, stop=True,
                         skip_group_check=True)

        # DVE/ACT probes: observe DMA lanes for tensors those engines touch
        # first, and the ACT const-bias table lane, one wait at a time.
        dve_probe = const.tile([1, 4], f32)
        nc.vector.tensor_copy(out=dve_probe[:, 0:1], in_=cT_sb[0:1, 0:1])
        nc.vector.tensor_copy(out=dve_probe[:, 1:2], in_=vit_sb[0:1, 0:1])
        act_probe = const.tile([1, 4], f32)
        nc.scalar.activation(act_probe[:, 0:1], dve_probe[:, 2:3],
                             mybir.ActivationFunctionType.Sigmoid)
        nc.scalar.activation(act_probe[:, 1:2], cT_sb[0:1, 0:1],
                             mybir.ActivationFunctionType.Tanh)

        # ---- stage 1: xg = x_aug^T.T @ wih (accumulate 3 row-chunks) ----
        for d in range(2):
            for gc in range(NG):
                col = d * GW + gc * 128
                for half in range(2):
                    pt = ps1.tile([128, 512], f32)
                    for ec, (r0, nr) in enumerate(rows):
                        nc.tensor.matmul(out=pt[:],
                                         lhsT=wih_sb[ec][:nr, col:col + 128],
                                         rhs=xT_sb[ec][:nr, half * 512:(half + 1) * 512],
                                         start=(ec == 0), stop=(ec == 2))
                    base = (d * NG + gc) * BC * T
                    nc.vector.tensor_copy(out=xg_sb[:, base + half * 512: base + (half + 1) * 512],
                                          in_=pt[:])

        # ---- stage 2: bidirectional LSTM recurrence ----
        for s in range(T):
            for d in range(2):
                t = s if d == 0 else T - 1 - s
                rd = t if d == 0 else t + 2   # prev-state slot
                wr = t + 1                     # written slot
                pg = ps2.tile([128, 32], f32)
                for gc in range(NG):
                    col = d * GW + gc * 128
                    for c in range(2):
                        nc.tensor.matmul(out=pg[:, gc * 4:(gc + 1) * 4],
                                         lhsT=whh_sb[c][:, col:col + 128],
                                         rhs=hsv[:, d, c, :, rd],
                                         start=(c == 0), stop=(c == 1))
                gsb = sbuf.tile([128, 32], f32, tag="gsb")
                nc.vector.tensor_add(out=gsb[:].rearrange("p (g b) -> p g b", g=NG),
                                     in0=pg[:].rearrange("p (g b) -> p g b", g=NG),
                                     in1=xgv[:, d, :, :, t])
                act = sbuf.tile([128, 24], f32, tag="act")
                import concourse.mybir as mybir
                nc.scalar.activation(act[:], gsb[:, 0:24], mybir.ActivationFunctionType.Sigmoid)
                tgg = sbuf.tile([128, 8], f32, tag="tgg")
                nc.scalar.activation(tgg[:], gsb[:, 24:32], mybir.ActivationFunctionType.Tanh)
                t1 = sbuf.tile([128, 8], f32, tag="t1")
                nc.vector.tensor_mul(out=t1[:], in0=act[:, 0:8], in1=tgg[:])
                t2 = sbuf.tile([128, 8], f32, tag="t2")
                nc.vector.tensor_mul(out=t2[:], in0=act[:, 8:16], in1=cT_sb[:, d * 8:d * 8 + 8])
                nc.vector.tensor_add(out=cT_sb[:, d * 8:d * 8 + 8], in0=t1[:], in1=t2[:])
                tcc = sbuf.tile([128, 8], f32, tag="tcc")
                nc.scalar.activation(tcc[:], cT_sb[:, d * 8:d * 8 + 8],
                                     mybir.ActivationFunctionType.Tanh)
                nc.vector.tensor_mul(out=hsv[:, d, :, :, wr],
                                     in0=act[:, 16:24].rearrange("p (c b) -> p c b", c=2),
                                     in1=tcc[:].rearrange("p (c b) -> p c b", c=2))

        # ---- stage 3: feats^T = wout.T @ lstm_out^T (+bias), token order (t, b) ----
        import concourse.mybir as mybir
        featsT = const.tile([12, BC * T], f32)
        for half in range(2):  # 512 cols = 128 t-values x 4 b (t-major)
            pf = ps3.tile([12, 512], f32)
            for d in range(2):
                for c in range(2):
                    # rhs [128, (t 128, b 4)]: i slot = t+1
                    hv = hsv[:, d, c, :, :].rearrange("p b i -> p i b")
                    nc.tensor.matmul(out=pf[:],
                                     lhsT=wout_sb[:, (d * 2 + c) * K:(d * 2 + c + 1) * K],
                                     rhs=hv[:, half * 128 + 1: half * 128 + 129, :],
                                     start=(d == 0 and c == 0), stop=False)
            nc.tensor.matmul(out=pf[:], lhsT=wout_b[:],
                             rhs=ones_sb[:], start=False, stop=True)
            nc.vector.tensor_copy(out=featsT[:, half * 512:(half + 1) * 512], in_=pf[:])
        # transpose each step's [12, 4] block -> [4, 12] at partitions 0-3
        for t in range(T):
            tp = ps3.tile([BC, K], f32, tag="tp")
            nc.tensor.transpose(out=tp[:], in_=featsT[:, t * 4:(t + 1) * 4],
                                identity=ident[0:12, 0:12])
            nc.vector.tensor_copy(out=feats_sb[:, t * K:(t + 1) * K], in_=tp[:])

        # ---- stage 4: viterbi ----
        import concourse.bass as bass_mod
        nc.vector.tensor_copy(out=pre_sb[:], in_=vit_sb[:, 300:312])
        trans_v = vit_sb[:, 0:144].rearrange("p (j k) -> p j k", j=K)
        revk_v = vit_sb[:, 144:288].rearrange("p (j k) -> p j k", j=K)

        def bcast_k(ap):  # [4, 12] -> [4, 12, 12] broadcasting over last (k)
            return bass_mod.AP(ap.tensor, ap.offset, [ap.ap[0], ap.ap[1], [0, K]])

        def bcast_j(ap):  # [4, 12] -> [4, 12(j broadcast), 12]
            return bass_mod.AP(ap.tensor, ap.offset, [ap.ap[0], [0, K], ap.ap[1]])

        for t in range(T):
            ft = feats_sb[:, t * K:(t + 1) * K]
            t1v = sbuf.tile([BC, 144], f32, tag="vt1")
            t1r = t1v[:].rearrange("p (j k) -> p j k", j=K)
            nc.vector.tensor_add(out=t1r, in0=trans_v, in1=bcast_j(pre_sb[:]))
            mm = sbuf.tile([BC, K], f32, tag="vmm")
            nc.vector.tensor_reduce(out=mm[:], in_=t1r, axis=mybir.AxisListType.X,
                                    op=mybir.AluOpType.max)
            eqv = sbuf.tile([BC, 144], f32, tag="veq")
            eqr = eqv[:].rearrange("p (j k) -> p j k", j=K)
            nc.vector.tensor_tensor(out=eqr, in0=t1r, in1=bcast_k(mm[:]),
                                    op=mybir.AluOpType.is_equal)
            nc.vector.tensor_mul(out=eqr, in0=eqr, in1=revk_v)
            nc.vector.tensor_reduce(out=bp_sb[:, t * K:(t + 1) * K], in_=eqr,
                                    axis=mybir.AxisListType.X, op=mybir.AluOpType.max)
            nc.vector.tensor_add(out=pre_sb[:], in0=mm[:], in1=ft)
            if t == T - 1:
                nc.vector.tensor_add(out=pre_sb[:], in0=pre_sb[:], in1=vit_sb[:, 288:300])

        nc.sync.dma_start(out=out_bp[:], in_=bp_sb[:])
        nc.sync.dma_start(out=out_pre[:], in_=pre_sb[:])

    # Several ISA structs (LDWEIGHTS, TT) hold a single sync wait. Drop
    # self-engine waits on multi-wait compute instructions: each engine
    # executes its stream in order, so a wait on its own semaphore is
    # satisfied by program order.
    eng_prefix = {"PE": "PE", "DVE": "DVE", "Activation": "Activation", "Pool": "Pool"}
    for bb in nc.main_func.blocks:
        for inst in bb.instructions:
            si = inst.sync_info
            if si is None or len(si.on_wait) <= 1:
                continue
            if type(inst).__name__ in ("InstDrain", "InstNop"):
                continue  # handled by the splitting pass below
            if type(inst).__name__ == "InstDMACopy":
                # HWDGE rings are FIFO per issuing engine and sem updates are
                # commutative adds; the same-lane ordering wait is redundant.
                keep = [w for w in si.on_wait if not w.ant_name.startswith("DMAHW")]
                if 0 < len(keep) <= 1:
                    si.on_wait = keep
                    inst.sync_info = si
                continue
            pfx = eng_prefix.get(str(getattr(inst, "engine", "")).split(".")[-1].replace("EngineType.", ""))
            eng = str(getattr(inst, "engine", ""))
            pfx = ("PE" if "PE" in eng else "DVE" if "DVE" in eng else
                   "Activation" if "Activation" in eng else "Pool" if "Pool" in eng else None)
            if pfx is None:
                continue
            keep = [w for w in si.on_wait if not w.ant_name.startswith(pfx)]
            if len(keep) != len(si.on_wait) and len(keep) <= 1:
                si.on_wait = keep
                inst.sync_info = si
    # Split any instruction still carrying >1 wait (e.g. the tail drain, 12
    # waits) into a chain of single-wait drain clones ahead of it.
    import copy as _copy
    fix_id = [0]
    for bb in nc.main_func.blocks:
        il = bb.instructions
        idx = 0
        while idx < len(il):
            inst = il[idx]
            si = inst.sync_info
            if si is not None and len(si.on_wait) > 1 and                     type(inst).__name__ == "InstDrain":
                waits = list(si.on_wait)
                for w in waits[:-1]:
                    cl = _copy.deepcopy(inst)
                    csi = cl.sync_info
                    csi.on_wait = [w]
                    csi.on_update = []
                    cl.sync_info = csi
                    cl.name = f"I-waitfix{fix_id[0]}"
                    fix_id[0] += 1
                    il.insert(idx, cl)
                    idx += 1
                si.on_wait = [waits[-1]]
                inst.sync_info = si
            idx += 1
    return nc


def _prep_core(i, x, wih_a, whh_a, st_pack, wout_a, vit_a):
    b0 = i * BC
    xt = x[b0:b0 + BC].reshape(BC * T, E).T.astype(np.float32)
    xt = np.concatenate([xt, np.ones((1, BC * T), np.float32)], 0)
    return {
        "xT": np.ascontiguousarray(xt),
        "wih": wih_a, "whhT": whh_a, "stT": st_pack[i],
        "woutT": wout_a, "vit": vit_a,
    }


def kernel(sentence, lens, emb, Wih_f, Whh_f, bih_f, bhh_f,
           Wih_b, Whh_b, bih_b, bhh_b, h0, c0, Wout, bout, transitions):
    sentence = np.asarray(sentence)
    emb = np.asarray(emb, np.float32)
    x = emb[sentence.astype(np.int64)]  # [B, T, E]

    perm = np.concatenate([np.arange(0, 256), np.arange(256, 512),
                           np.arange(768, 1024), np.arange(512, 768)])
    def prep_dir(Wih, Whh, bih, bhh):
        wih_t = np.asarray(Wih, np.float32)[perm].T                      # [300, 1024]
        bias = (np.asarray(bih, np.float32) + np.asarray(bhh, np.float32))[perm]
        wih_aug = np.concatenate([wih_t, bias[None, :]], 0)              # [301, 1024]
        whh_t = np.asarray(Whh, np.float32)[perm].T                      # [256, 1024]
        return wih_aug, whh_t

    wf, hf = prep_dir(Wih_f, Whh_f, bih_f, bhh_f)
    wb, hb = prep_dir(Wih_b, Whh_b, bih_b, bhh_b)
    wih_a = np.ascontiguousarray(np.concatenate([wf, wb], 1))            # [301, 2048]
    whh_a = np.ascontiguousarray(np.concatenate([hf, hb], 1))            # [256, 2048]

    h0 = np.asarray(h0, np.float32); c0 = np.asarray(c0, np.float32)
    st_pack = []
    for i in range(NCORES):
        b0 = i * BC
        def pack(z):  # [2, BC, 256] -> [128, (d, c, b)]
            return z[:, b0:b0 + BC].reshape(2, BC, 2, 128).transpose(3, 0, 2, 1).reshape(128, 16)
        st_pack.append(np.ascontiguousarray(
            np.concatenate([pack(h0), pack(c0)], 1).astype(np.float32)))

    wout_a = np.ascontiguousarray(np.concatenate(
        [np.asarray(Wout, np.float32).T, np.asarray(bout, np.float32)[None, :]], 0))

    trans = np.asarray(transitions, np.float32)
    vit_row = np.concatenate([
        trans.reshape(-1),
        np.tile((K - 1) - np.arange(K, dtype=np.float32), K),
        trans[STOP],
        np.where(np.arange(K) == START, 0.0, -10000.0).astype(np.float32),
    ]).astype(np.float32)
    vit_a = np.ascontiguousarray(np.tile(vit_row[None, :], (BC, 1)))

    if "nc" not in _CACHE:
        _CACHE["nc"] = _build_nc()
    nc = _CACHE["nc"]

    in_maps = [_prep_core(i, x, wih_a, whh_a, st_pack, wout_a, vit_a)
               for i in range(NCORES)]
    import os, time as _time
    from concourse.bass_utils import run_bass_kernel_spmd
    t0 = _time.time()
    if os.environ.get("BASS_PROFILE"):
        try:
            res = run_bass_kernel_spmd(nc, in_maps, list(range(NCORES)), trace=True)
            if res.exec_time_ns is not None:
                _CACHE["exec_ns"] = res.exec_time_ns
        except Exception:
            res = run_bass_kernel_spmd(nc, in_maps, list(range(NCORES)))
    else:
        res = run_bass_kernel_spmd(nc, in_maps, list(range(NCORES)))
    _CACHE["run_wall_s"] = _time.time() - t0

    pre_all = np.concatenate([res.results[i]["out_pre"] for i in range(NCORES)], 0)  # [32, 12]
    bp_all = np.concatenate([np.asarray(res.results[i]["out_bp"]).reshape(BC, T, K)
                             for i in range(NCORES)], 0)                             # [32, T, 12]
    bp = ((K - 1) - bp_all).astype(np.int64)                                         # argmax-first indices

    scores = pre_all.max(axis=1).astype(np.float32)
    idx = pre_all.argmax(axis=1)
    ar = np.arange(B)
    ys = np.empty((T, B), np.int64)
    cur = idx.copy()
    for t in range(T - 1, -1, -1):
        cur = bp[ar, t, cur]
        ys[t] = cur
    paths = np.concatenate([ys[1:], idx[None, :]], 0).T.astype(np.int32)
    return scores, paths
